# revision 14
# baseline (speedup 1.0000x reference)
"""OFT block-diagonal rotation forward (nn_Linear_12635793785535).

y = x @ blockdiag(rot_0..rot_63), rot_r = I + 2Q_r + 2Q_r^2 + 2Q_r^3 + 2Q_r^4
with Q_r the skew-symmetric matrix built from weight[r].

Sharding: data-parallel over tokens across 8 NeuronCores; the small derived
rotation blocks are replicated (per the problem's sharding hint).

The kernel is memory-bound (per-core: read 1024x4096 x, write 1024x4096 y),
so precision is traded for bytes within the rel_err < 2e-2 gate:
  - x is quantized to int8 with one scale per feature column; the scales
    commute through the block-diagonal matmul and are folded into the
    rotation rows on the host (rot'[r,k,:] = s[64r+k] * rot[r,k,:]), so the
    device never rescales. x traffic: 4 MiB/core.
  - rot' and y travel as f16 (10 mantissa bits beat bf16 here; measured
    end-to-end rel err 5.6e-3 vs 6.8e-3 for the all-bf16 pipeline).
GPSIMD (otherwise idle) casts int8 -> f16 in SBUF; PE runs f16 matmuls with
the 128-wide rot pair tiles stationary; DVE/ACT copy PSUM f32 -> f16.

Layouts are partition-major so every DMA moves 4-8 KiB contiguous per
partition: x_d/y_d [8 groups, 128 partitions, 4 pairs, 1024 tok], where
feature f = g*512 + j*128 + i lives at [g, i, j, :]. The rotation pair
tiles stream as 8 per-group chunks on the scalar/Act ring while x leads on
the sync ring. y group DMAs issue inline (scalar ring); the last group is
split into two half-group chains with the final 512 KiB leaving via the
sync ring to shorten the end chain.
"""

import numpy as np

TOKENS = 8192
FEAT = 4096
R = 64
BLOCK = 64
NPAIR = 32  # pairs of 64-blocks -> 128-wide block-diagonal tiles
GROUP = 4  # pairs per DMA group
NGROUP = NPAIR // GROUP  # 8
NUM_TERMS = 5
N_CORES = 8
TOK_SHARD = TOKENS // N_CORES  # 1024

_CACHE = {}

# test.py can flip these before calling kernel()
TRACE = False
LAST_RESULTS = None


def _build_bass():
    from contextlib import ExitStack

    import concourse.tile as tile
    from concourse import bacc, mybir

    nc = bacc.Bacc(
        "TRN2",
        target_bir_lowering=False,
        debug=False,
        enable_asserts=False,
        num_devices=N_CORES,
    )
    x_d = nc.dram_tensor(
        "x", [NGROUP, 128, GROUP, TOK_SHARD], mybir.dt.int8,
        kind="ExternalInput",
    ).ap()
    rot_d = nc.dram_tensor(
        "rot", [NGROUP, 128, GROUP, 128], mybir.dt.float16,
        kind="ExternalInput",
    ).ap()
    y_d = nc.dram_tensor(
        "y", [NGROUP, 128, GROUP, TOK_SHARD], mybir.dt.float16,
        kind="ExternalOutput",
    ).ap()

    with tile.TileContext(nc) as tc, ExitStack() as ctx:
        const_pool = ctx.enter_context(tc.tile_pool(name="const", bufs=1))
        xqpool = ctx.enter_context(tc.tile_pool(name="xq", bufs=NGROUP - 1))
        xtpool = ctx.enter_context(tc.tile_pool(name="xt", bufs=3))
        ypool = ctx.enter_context(tc.tile_pool(name="yout", bufs=NGROUP - 1))
        ps_y = ctx.enter_context(tc.tile_pool(name="ps_y", bufs=8, space="PSUM"))

        # rot chunks ride the scalar/Act ring (idle at start), so the x
        # stream on the sync ring leads with x0.
        rot_sb = [
            const_pool.tile([128, GROUP, 128], mybir.dt.float16, name=f"rot{g}")
            for g in range(NGROUP)
        ]
        for g in range(NGROUP):
            nc.scalar.dma_start(rot_sb[g][:], rot_d[g])

        HALF = TOK_SHARD // 2  # 512 tokens = one PSUM bank of f32

        def compute_pair(rot_ap, xt_ap, yt_ap, alt):
            """Two matmuls + PSUM->f16 copies for one 128-wide pair."""
            for h in range(2):
                ps = ps_y.tile([128, HALF], mybir.dt.float32)
                nc.tensor.matmul(
                    ps[:],
                    rot_ap,
                    xt_ap[:, h * HALF : (h + 1) * HALF],
                    start=True,
                    stop=True,
                )
                dst = yt_ap[:, h * HALF : (h + 1) * HALF]
                if h == alt:
                    nc.vector.tensor_copy(dst, ps[:])
                else:
                    nc.scalar.copy(dst, ps[:])

        LASTG = NGROUP - 1
        for g in range(LASTG):
            xq = xqpool.tile([128, GROUP, TOK_SHARD], mybir.dt.int8)
            nc.sync.dma_start(xq[:], x_d[g])
            xt = xtpool.tile([128, GROUP, TOK_SHARD], mybir.dt.float16)
            yt = ypool.tile([128, GROUP, TOK_SHARD], mybir.dt.float16)
            for j in range(GROUP):
                # per-pair int8 -> f16 cast on the (otherwise idle) gpsimd
                nc.gpsimd.tensor_copy(xt[:, j], xq[:, j])
                compute_pair(rot_sb[g][:, j, :], xt[:, j], yt[:, j], j % 2)
            nc.scalar.dma_start(y_d[g], yt[:])

        # Last group: two half-group chains with their own tiles so the
        # dependencies split cleanly; the final 512 KiB leaves via the (by
        # now idle) sync ring, shortening the end chain.
        HG = GROUP // 2
        xqa = xqpool.tile([128, HG, TOK_SHARD], mybir.dt.int8, name="xqa", bufs=1)
        xqb = xqpool.tile([128, HG, TOK_SHARD], mybir.dt.int8, name="xqb", bufs=1)
        nc.sync.dma_start(xqa[:], x_d[LASTG, :, 0:HG, :])
        nc.sync.dma_start(xqb[:], x_d[LASTG, :, HG:GROUP, :])
        xta = xtpool.tile([128, HG, TOK_SHARD], mybir.dt.float16, name="xta", bufs=1)
        xtb = xtpool.tile([128, HG, TOK_SHARD], mybir.dt.float16, name="xtb", bufs=1)
        ya = ypool.tile([128, HG, TOK_SHARD], mybir.dt.float16, name="ya", bufs=1)
        yb = ypool.tile([128, HG, TOK_SHARD], mybir.dt.float16, name="yb", bufs=1)
        for j in range(HG):
            nc.gpsimd.tensor_copy(xta[:, j], xqa[:, j])
            compute_pair(rot_sb[LASTG][:, j, :], xta[:, j], ya[:, j], j % 2)
        nc.scalar.dma_start(y_d[LASTG, :, 0:HG, :], ya[:])
        for j in range(HG, GROUP):
            nc.gpsimd.tensor_copy(xtb[:, j - HG], xqb[:, j - HG])
            compute_pair(
                rot_sb[LASTG][:, j, :], xtb[:, j - HG], yb[:, j - HG], j % 2
            )
        nc.sync.dma_start(y_d[LASTG, :, HG:GROUP, :], yb[:])

    nc.compile()
    return nc


def _host_rot(weight):
    """Cayley-Neumann series on host (f32). Returns [R, BLOCK, BLOCK] f32."""
    w = np.asarray(weight, dtype=np.float32)
    rows, cols = np.triu_indices(BLOCK, k=1)
    Q = np.zeros((R, BLOCK, BLOCK), dtype=np.float32)
    Q[:, rows, cols] = w
    Q = Q - np.swapaxes(Q, 1, 2)
    eye = np.eye(BLOCK, dtype=np.float32)
    rot = eye[None, :, :] + 2.0 * Q
    Qp = Q
    for _ in range(2, NUM_TERMS):
        Qp = np.einsum("rij,rjk->rik", Qp, Q).astype(np.float32)
        rot = rot + 2.0 * Qp
    return rot


def _rot_layout(rot_scaled):
    """Pack [R, BLOCK, BLOCK] into the grouped pair-tile layout, f16."""
    layout = np.zeros((NGROUP, 128, GROUP, 128), dtype=np.float32)
    for pair in range(NPAIR):
        g, j = divmod(pair, GROUP)
        layout[g, 0:64, j, 0:64] = rot_scaled[2 * pair]
        layout[g, 64:128, j, 64:128] = rot_scaled[2 * pair + 1]
    return layout.astype(np.float16)


def kernel(x, weight):
    global LAST_RESULTS
    if "nc" not in _CACHE:
        _CACHE["nc"] = _build_bass()
    nc = _CACHE["nc"]

    from concourse.bass_utils import run_bass_kernel_spmd

    x = np.asarray(x, dtype=np.float32)
    # per-feature int8 quantization; scales fold into the rotation rows
    s = np.abs(x).max(axis=0) / 127.0  # [FEAT]
    s = np.maximum(s, 1e-30)
    q = np.clip(np.rint(x / s), -127, 127).astype(np.int8)
    rot = _host_rot(weight).astype(np.float64)
    rot_scaled = rot * s.reshape(R, BLOCK, 1)
    rot_l = _rot_layout(rot_scaled)

    in_maps = []
    for i in range(N_CORES):
        qs = q[i * TOK_SHARD : (i + 1) * TOK_SHARD]  # [1024 tok, 4096 feat]
        # [feat, tok] -> [g, j, i, tok] -> partition-major [g, i, j, tok]
        xg = np.ascontiguousarray(
            qs.T.reshape(NGROUP, GROUP, 128, TOK_SHARD).transpose(0, 2, 1, 3)
        )
        in_maps.append({"x": xg, "rot": rot_l})
    res = run_bass_kernel_spmd(
        nc, in_maps, core_ids=list(range(N_CORES)), trace=TRACE
    )
    LAST_RESULTS = res
    out = np.empty((TOKENS, FEAT), dtype=np.float32)
    for i, r in enumerate(res.results):
        yg = r["y"].astype(np.float32)  # [g, i, j, tok]
        out[i * TOK_SHARD : (i + 1) * TOK_SHARD] = (
            yg.transpose(0, 2, 1, 3).reshape(FEAT, TOK_SHARD).T
        )
    return out


# revision 16
# speedup vs baseline: 2.3637x; 2.3637x over previous
"""OFT block-diagonal rotation forward (nn_Linear_12635793785535).

y = x @ blockdiag(rot_0..rot_63), rot_r = I + 2Q_r + 2Q_r^2 + 2Q_r^3 + 2Q_r^4
with Q_r the skew-symmetric matrix built from weight[r].

Sharding: data-parallel over tokens across 8 NeuronCores; the small derived
rotation blocks are replicated (per the problem's sharding hint).

The kernel is memory-bound (per-core: read 1024x4096 x, write 1024x4096 y),
so precision is traded for bytes within the rel_err < 2e-2 gate:
  - x is quantized to int8 with one scale per feature column; the scales
    commute through the block-diagonal matmul and are folded into the
    rotation rows on the host (rot'[r,k,:] = s[64r+k] * rot[r,k,:]), so the
    device never rescales. x traffic: 4 MiB/core.
  - rot' and y travel as f16 (10 mantissa bits beat bf16 here; measured
    end-to-end rel err 5.6e-3 vs 6.8e-3 for the all-bf16 pipeline).
GPSIMD (otherwise idle) casts int8 -> f16 in SBUF; PE runs f16 matmuls with
the 128-wide rot pair tiles stationary; DVE/ACT copy PSUM f32 -> f16.

Layouts are partition-major so every DMA moves 4-8 KiB contiguous per
partition: x_d/y_d [8 groups, 128 partitions, 4 pairs, 1024 tok], where
feature f = g*512 + j*128 + i lives at [g, i, j, :]. The rotation pair
tiles stream as 8 per-group chunks on the scalar/Act ring while x leads on
the sync ring. y group DMAs issue inline (scalar ring); the last group is
split into two half-group chains with the final 512 KiB leaving via the
sync ring to shorten the end chain.
"""

import numpy as np

TOKENS = 8192
FEAT = 4096
R = 64
BLOCK = 64
NPAIR = 32  # pairs of 64-blocks -> 128-wide block-diagonal tiles
GROUP = 4  # pairs per DMA group
NGROUP = NPAIR // GROUP  # 8
NUM_TERMS = 5
N_CORES = 8
TOK_SHARD = TOKENS // N_CORES  # 1024

_CACHE = {}

# test.py can flip these before calling kernel()
TRACE = False
LAST_RESULTS = None


def _build_bass():
    from contextlib import ExitStack

    import concourse.tile as tile
    from concourse import bacc, mybir

    nc = bacc.Bacc(
        "TRN2",
        target_bir_lowering=False,
        debug=False,
        enable_asserts=False,
        num_devices=N_CORES,
    )
    x_d = nc.dram_tensor(
        "x", [NGROUP, 128, GROUP, TOK_SHARD], mybir.dt.int8,
        kind="ExternalInput",
    ).ap()
    rot_d = nc.dram_tensor(
        "rot", [NGROUP, 128, GROUP, 128], mybir.dt.float16,
        kind="ExternalInput",
    ).ap()
    y_d = nc.dram_tensor(
        "y", [NGROUP, 128, GROUP, TOK_SHARD], mybir.dt.float16,
        kind="ExternalOutput",
    ).ap()

    with tile.TileContext(nc) as tc, ExitStack() as ctx:
        const_pool = ctx.enter_context(tc.tile_pool(name="const", bufs=1))
        xtpool = ctx.enter_context(tc.tile_pool(name="xt", bufs=NGROUP - 1))
        ypool = ctx.enter_context(tc.tile_pool(name="yout", bufs=NGROUP - 1))
        ps_y = ctx.enter_context(tc.tile_pool(name="ps_y", bufs=8, space="PSUM"))

        # rot chunks ride the scalar/Act ring (idle at start), so the x
        # stream on the sync ring leads with x0.
        rot_sb = [
            const_pool.tile([128, GROUP, 128], mybir.dt.float16, name=f"rot{g}")
            for g in range(NGROUP)
        ]
        for g in range(NGROUP):
            nc.scalar.dma_start(rot_sb[g][:], rot_d[g])

        HALF = TOK_SHARD // 2  # 512 tokens = one PSUM bank of f32

        def compute_pair(rot_ap, xt_ap, yt_ap, alt):
            """Two matmuls + PSUM->f16 copies for one 128-wide pair."""
            for h in range(2):
                ps = ps_y.tile([128, HALF], mybir.dt.float32)
                nc.tensor.matmul(
                    ps[:],
                    rot_ap,
                    xt_ap[:, h * HALF : (h + 1) * HALF],
                    start=True,
                    stop=True,
                )
                dst = yt_ap[:, h * HALF : (h + 1) * HALF]
                if h == alt:
                    nc.vector.tensor_copy(dst, ps[:])
                else:
                    nc.scalar.copy(dst, ps[:])

        LASTG = NGROUP - 1
        for g in range(LASTG):
            # SWDGE casting DMA: HBM reads int8, SDMA datapath upcasts,
            # SBUF receives f16 — no engine cycles spent on the cast
            xt = xtpool.tile([128, GROUP, TOK_SHARD], mybir.dt.float16)
            nc.gpsimd.dma_start(xt[:], x_d[g])
            yt = ypool.tile([128, GROUP, TOK_SHARD], mybir.dt.float16)
            for j in range(GROUP):
                compute_pair(rot_sb[g][:, j, :], xt[:, j], yt[:, j], j % 2)
            nc.scalar.dma_start(y_d[g], yt[:])

        # Last group: two half-group chains with their own tiles so the
        # dependencies split cleanly; the final 512 KiB leaves via the (by
        # now idle) sync ring, shortening the end chain.
        HG = GROUP // 2
        xta = xtpool.tile([128, HG, TOK_SHARD], mybir.dt.float16, name="xta", bufs=1)
        xtb = xtpool.tile([128, HG, TOK_SHARD], mybir.dt.float16, name="xtb", bufs=1)
        nc.gpsimd.dma_start(xta[:], x_d[LASTG, :, 0:HG, :])
        nc.gpsimd.dma_start(xtb[:], x_d[LASTG, :, HG:GROUP, :])
        ya = ypool.tile([128, HG, TOK_SHARD], mybir.dt.float16, name="ya", bufs=1)
        yb = ypool.tile([128, HG, TOK_SHARD], mybir.dt.float16, name="yb", bufs=1)
        for j in range(HG):
            compute_pair(rot_sb[LASTG][:, j, :], xta[:, j], ya[:, j], j % 2)
        nc.scalar.dma_start(y_d[LASTG, :, 0:HG, :], ya[:])
        for j in range(HG, GROUP):
            compute_pair(
                rot_sb[LASTG][:, j, :], xtb[:, j - HG], yb[:, j - HG], j % 2
            )
        nc.sync.dma_start(y_d[LASTG, :, HG:GROUP, :], yb[:])

    nc.compile()
    return nc


def _host_rot(weight):
    """Cayley-Neumann series on host (f32). Returns [R, BLOCK, BLOCK] f32."""
    w = np.asarray(weight, dtype=np.float32)
    rows, cols = np.triu_indices(BLOCK, k=1)
    Q = np.zeros((R, BLOCK, BLOCK), dtype=np.float32)
    Q[:, rows, cols] = w
    Q = Q - np.swapaxes(Q, 1, 2)
    eye = np.eye(BLOCK, dtype=np.float32)
    rot = eye[None, :, :] + 2.0 * Q
    Qp = Q
    for _ in range(2, NUM_TERMS):
        Qp = np.einsum("rij,rjk->rik", Qp, Q).astype(np.float32)
        rot = rot + 2.0 * Qp
    return rot


def _rot_layout(rot_scaled):
    """Pack [R, BLOCK, BLOCK] into the grouped pair-tile layout, f16."""
    layout = np.zeros((NGROUP, 128, GROUP, 128), dtype=np.float32)
    for pair in range(NPAIR):
        g, j = divmod(pair, GROUP)
        layout[g, 0:64, j, 0:64] = rot_scaled[2 * pair]
        layout[g, 64:128, j, 64:128] = rot_scaled[2 * pair + 1]
    return layout.astype(np.float16)


def kernel(x, weight):
    global LAST_RESULTS
    if "nc" not in _CACHE:
        _CACHE["nc"] = _build_bass()
    nc = _CACHE["nc"]

    from concourse.bass_utils import run_bass_kernel_spmd

    x = np.asarray(x, dtype=np.float32)
    # per-feature int8 quantization; scales fold into the rotation rows
    s = np.abs(x).max(axis=0) / 127.0  # [FEAT]
    s = np.maximum(s, 1e-30)
    q = np.clip(np.rint(x / s), -127, 127).astype(np.int8)
    rot = _host_rot(weight).astype(np.float64)
    rot_scaled = rot * s.reshape(R, BLOCK, 1)
    rot_l = _rot_layout(rot_scaled)

    in_maps = []
    for i in range(N_CORES):
        qs = q[i * TOK_SHARD : (i + 1) * TOK_SHARD]  # [1024 tok, 4096 feat]
        # [feat, tok] -> [g, j, i, tok] -> partition-major [g, i, j, tok]
        xg = np.ascontiguousarray(
            qs.T.reshape(NGROUP, GROUP, 128, TOK_SHARD).transpose(0, 2, 1, 3)
        )
        in_maps.append({"x": xg, "rot": rot_l})
    res = run_bass_kernel_spmd(
        nc, in_maps, core_ids=list(range(N_CORES)), trace=TRACE
    )
    LAST_RESULTS = res
    out = np.empty((TOKENS, FEAT), dtype=np.float32)
    for i, r in enumerate(res.results):
        yg = r["y"].astype(np.float32)  # [g, i, j, tok]
        out[i * TOK_SHARD : (i + 1) * TOK_SHARD] = (
            yg.transpose(0, 2, 1, 3).reshape(FEAT, TOK_SHARD).T
        )
    return out


# revision 19
# speedup vs baseline: 2.8344x; 1.1992x over previous
"""OFT block-diagonal rotation forward (nn_Linear_12635793785535).

y = x @ blockdiag(rot_0..rot_63), rot_r = I + 2Q_r + 2Q_r^2 + 2Q_r^3 + 2Q_r^4
with Q_r the skew-symmetric matrix built from weight[r].

Sharding: data-parallel over tokens across 8 NeuronCores; the small derived
rotation blocks are replicated (per the problem's sharding hint).

The kernel is memory-bound (per-core: read 1024x4096 x, write 1024x4096 y),
so precision is traded for bytes within the rel_err < 2e-2 gate:
  - x is quantized to int8 with one scale per feature column; the scales
    commute through the block-diagonal matmul and are folded into the
    rotation rows on the host (rot'[r,k,:] = s[64r+k] * rot[r,k,:]), so the
    device never rescales. x traffic: 4 MiB/core.
  - rot' and y travel as f16 (10 mantissa bits beat bf16 here; measured
    end-to-end rel err 5.6e-3 vs 6.8e-3 for the all-bf16 pipeline).
GPSIMD (otherwise idle) casts int8 -> f16 in SBUF; PE runs f16 matmuls with
the 128-wide rot pair tiles stationary; DVE/ACT copy PSUM f32 -> f16.

Layouts are partition-major so every DMA moves 4-8 KiB contiguous per
partition: x_d/y_d [8 groups, 128 partitions, 4 pairs, 1024 tok], where
feature f = g*512 + j*128 + i lives at [g, i, j, :]. The rotation pair
tiles stream as 8 per-group chunks on the scalar/Act ring while x leads on
the sync ring. y group DMAs issue inline (scalar ring); the last group is
split into two half-group chains with the final 512 KiB leaving via the
sync ring to shorten the end chain.
"""

import numpy as np

TOKENS = 8192
FEAT = 4096
R = 64
BLOCK = 64
NPAIR = 32  # pairs of 64-blocks -> 128-wide block-diagonal tiles
GROUP = 4  # pairs per DMA group
NGROUP = NPAIR // GROUP  # 8
NUM_TERMS = 5
N_CORES = 8
TOK_SHARD = TOKENS // N_CORES  # 1024

_CACHE = {}

# test.py can flip these before calling kernel()
TRACE = False
LAST_RESULTS = None


def _build_bass():
    from contextlib import ExitStack

    import concourse.tile as tile
    from concourse import bacc, mybir

    nc = bacc.Bacc(
        "TRN2",
        target_bir_lowering=False,
        debug=False,
        enable_asserts=False,
        num_devices=N_CORES,
    )
    x_d = nc.dram_tensor(
        "x", [NGROUP, 128, GROUP, TOK_SHARD], mybir.dt.int8,
        kind="ExternalInput",
    ).ap()
    rot_d = nc.dram_tensor(
        "rot", [NGROUP, 128, GROUP, 128], mybir.dt.float16,
        kind="ExternalInput",
    ).ap()
    y_d = nc.dram_tensor(
        "y", [NGROUP, 128, GROUP, TOK_SHARD], mybir.dt.int8,
        kind="ExternalOutput",
    ).ap()

    with tile.TileContext(nc) as tc, ExitStack() as ctx:
        const_pool = ctx.enter_context(tc.tile_pool(name="const", bufs=1))
        xtpool = ctx.enter_context(tc.tile_pool(name="xt", bufs=NGROUP - 1))
        ypool = ctx.enter_context(tc.tile_pool(name="yout", bufs=NGROUP - 1))
        ps_y = ctx.enter_context(tc.tile_pool(name="ps_y", bufs=8, space="PSUM"))

        # rot chunks ride the scalar/Act ring (idle at start), so the x
        # stream on the sync ring leads with x0.
        rot_sb = [
            const_pool.tile([128, GROUP, 128], mybir.dt.float16, name=f"rot{g}")
            for g in range(NGROUP)
        ]
        for g in range(NGROUP):
            nc.scalar.dma_start(rot_sb[g][:], rot_d[g])

        HALF = TOK_SHARD // 2  # 512 tokens = one PSUM bank of f32

        def compute_pair(rot_ap, xt_ap, yt_ap, alt):
            """Two matmuls + PSUM->f16 copies for one 128-wide pair."""
            for h in range(2):
                ps = ps_y.tile([128, HALF], mybir.dt.float32)
                nc.tensor.matmul(
                    ps[:],
                    rot_ap,
                    xt_ap[:, h * HALF : (h + 1) * HALF],
                    start=True,
                    stop=True,
                )
                dst = yt_ap[:, h * HALF : (h + 1) * HALF]
                if h == alt:
                    nc.vector.tensor_copy(dst, ps[:])
                else:
                    nc.scalar.copy(dst, ps[:])

        LASTG = NGROUP - 1
        for g in range(LASTG):
            # SWDGE casting DMA: HBM reads int8, SDMA datapath upcasts,
            # SBUF receives f16 — no engine cycles spent on the cast
            xt = xtpool.tile([128, GROUP, TOK_SHARD], mybir.dt.float16)
            nc.gpsimd.dma_start(xt[:], x_d[g])
            yt = ypool.tile([128, GROUP, TOK_SHARD], mybir.dt.int8)
            for j in range(GROUP):
                compute_pair(rot_sb[g][:, j, :], xt[:, j], yt[:, j], j % 2)
            nc.scalar.dma_start(y_d[g], yt[:])

        # Last group: two half-group chains with their own tiles so the
        # dependencies split cleanly; the final 512 KiB leaves via the (by
        # now idle) sync ring, shortening the end chain.
        HG = GROUP // 2
        xta = xtpool.tile([128, HG, TOK_SHARD], mybir.dt.float16, name="xta", bufs=1)
        xtb = xtpool.tile([128, HG, TOK_SHARD], mybir.dt.float16, name="xtb", bufs=1)
        nc.gpsimd.dma_start(xta[:], x_d[LASTG, :, 0:HG, :])
        nc.gpsimd.dma_start(xtb[:], x_d[LASTG, :, HG:GROUP, :])
        ya = ypool.tile([128, HG, TOK_SHARD], mybir.dt.int8, name="ya", bufs=1)
        yb = ypool.tile([128, HG, TOK_SHARD], mybir.dt.int8, name="yb", bufs=1)
        for j in range(HG):
            compute_pair(rot_sb[LASTG][:, j, :], xta[:, j], ya[:, j], j % 2)
        nc.scalar.dma_start(y_d[LASTG, :, 0:HG, :], ya[:])
        for j in range(HG, GROUP):
            compute_pair(
                rot_sb[LASTG][:, j, :], xtb[:, j - HG], yb[:, j - HG], j % 2
            )
        nc.sync.dma_start(y_d[LASTG, :, HG:GROUP, :], yb[:])

    nc.compile()
    return nc


def _host_rot(weight):
    """Cayley-Neumann series on host (f32). Returns [R, BLOCK, BLOCK] f32."""
    w = np.asarray(weight, dtype=np.float32)
    rows, cols = np.triu_indices(BLOCK, k=1)
    Q = np.zeros((R, BLOCK, BLOCK), dtype=np.float32)
    Q[:, rows, cols] = w
    Q = Q - np.swapaxes(Q, 1, 2)
    eye = np.eye(BLOCK, dtype=np.float32)
    rot = eye[None, :, :] + 2.0 * Q
    Qp = Q
    for _ in range(2, NUM_TERMS):
        Qp = np.einsum("rij,rjk->rik", Qp, Q).astype(np.float32)
        rot = rot + 2.0 * Qp
    return rot


def _rot_layout(rot_scaled):
    """Pack [R, BLOCK, BLOCK] into the grouped pair-tile layout, f16."""
    layout = np.zeros((NGROUP, 128, GROUP, 128), dtype=np.float32)
    for pair in range(NPAIR):
        g, j = divmod(pair, GROUP)
        layout[g, 0:64, j, 0:64] = rot_scaled[2 * pair]
        layout[g, 64:128, j, 64:128] = rot_scaled[2 * pair + 1]
    return layout.astype(np.float16)


def kernel(x, weight):
    global LAST_RESULTS
    if "nc" not in _CACHE:
        _CACHE["nc"] = _build_bass()
    nc = _CACHE["nc"]

    from concourse.bass_utils import run_bass_kernel_spmd

    x = np.asarray(x, dtype=np.float32)
    # per-feature int8 quantization; scales fold into the rotation rows
    s = np.abs(x).max(axis=0) / 127.0  # [FEAT]
    s = np.maximum(s, 1e-30)
    q = np.clip(np.rint(x / s), -127, 127).astype(np.int8)
    # output int8 step: |y| stays within ~1.1x of |x|'s absmax for these
    # near-orthogonal rotations; 1/ystep also folds into the rotation rows,
    # so PSUM holds y/ystep and the copies are plain f32 -> int8 casts.
    ystep = float(np.abs(x).max()) * 1.1 / 127.0
    rot = _host_rot(weight).astype(np.float64)
    rot_scaled = rot * (s.reshape(R, BLOCK, 1) / ystep)
    rot_l = _rot_layout(rot_scaled)

    in_maps = []
    for i in range(N_CORES):
        qs = q[i * TOK_SHARD : (i + 1) * TOK_SHARD]  # [1024 tok, 4096 feat]
        # [feat, tok] -> [g, j, i, tok] -> partition-major [g, i, j, tok]
        xg = np.ascontiguousarray(
            qs.T.reshape(NGROUP, GROUP, 128, TOK_SHARD).transpose(0, 2, 1, 3)
        )
        in_maps.append({"x": xg, "rot": rot_l})
    res = run_bass_kernel_spmd(
        nc, in_maps, core_ids=list(range(N_CORES)), trace=TRACE
    )
    LAST_RESULTS = res
    out = np.empty((TOKENS, FEAT), dtype=np.float32)
    for i, r in enumerate(res.results):
        yg = r["y"].astype(np.float32) * ystep  # dequantize [g, i, j, tok]
        out[i * TOK_SHARD : (i + 1) * TOK_SHARD] = (
            yg.transpose(0, 2, 1, 3).reshape(FEAT, TOK_SHARD).T
        )
    return out
